# revision 18
# baseline (speedup 1.0000x reference)
"""BitNet MLP (ternary-quantized SwiGLU) on 8 Trainium2 NeuronCores — v2.

Strategy: tensor-parallel over hidden_dim. Weights are ternarized on the
HOST (exactly matching the reference's jnp-on-CPU threshold) and uploaded
as fp8e4m3 — exact for {-1,0,+1} — which quarters weight DRAM traffic and
removes all on-device quantization work. The tensor engine runs mixed-dtype
matmuls (fp8 stationary weights x bf16 moving activations), verified exact
and full-speed on HW.

Phase A keeps the core's entire gate+up weight set resident in SBUF
(128 KB/partition) and streams x through ONCE in 512-token chunks,
producing hidden in bf16 to DRAM (one DMA per chunk). Phase B keeps the
whole down_proj shard resident (64 KB/partition), streams hidden back, and
ReduceScatters bf16 partial sums per chunk (overlapped with the next
chunk's matmuls), writing straight to the output chunk.
"""

import sys

sys.path.insert(0, "/opt/trn_rl_repo")

import numpy as np
import ml_dtypes

BF16 = ml_dtypes.bfloat16
FP8 = ml_dtypes.float8_e4m3
NCORES = 8
P = 128
TC = 512

_CACHE = {}


def _build(d, t_total, h_total, dim, scaled=False, with_collective=True,
           phases="AB"):
    import concourse.mybir as mybir
    import concourse.tile as tile
    from concourse import bacc

    f32 = mybir.dt.float32
    bf16 = mybir.dt.bfloat16
    fp8 = mybir.dt.float8e4

    h_local = h_total // NCORES
    dim_shard = dim // NCORES
    n_ko = d // P
    n_ht = h_local // P
    n_dt = dim // P
    n_tc = t_total // TC

    assert d % P == 0 and h_local % P == 0 and dim % P == 0
    assert t_total % TC == 0 and n_dt % 4 == 0
    wgu_cols = n_ht * n_ko * 2 * P
    wd_cols = n_dt * n_ht * P
    assert wgu_cols % 4 == 0

    nc = bacc.Bacc("TRN2", target_bir_lowering=False, debug=False)

    xT_e = nc.dram_tensor("xT", [d, t_total], bf16, kind="ExternalInput")
    wgu_e = nc.dram_tensor("wgu", [P, wgu_cols], fp8, kind="ExternalInput")
    wd_e = nc.dram_tensor("wd", [P, wd_cols], fp8, kind="ExternalInput")
    gs_e = nc.dram_tensor("gs", [h_local, 1], f32, kind="ExternalInput")
    us_e = nc.dram_tensor("us", [h_local, 1], f32, kind="ExternalInput")
    ds_e = nc.dram_tensor("ds", [dim_shard, 1], f32, kind="ExternalInput")
    out_e = nc.dram_tensor("out", [n_tc, dim_shard, TC], bf16,
                           kind="ExternalOutput")

    cc_outs = [
        nc.dram_tensor(f"cc_out{i}", [dim_shard, TC], bf16, kind="Internal")
        for i in range(n_tc)
    ]

    def wgu_sl(w, ht, ko, m):
        c = ((ht * n_ko + ko) * 2 + m) * P
        return w[:, c:c + P]

    def wd_sl(w, dt, ho):
        c = (dt * n_ht + ho) * P
        return w[:, c:c + P]

    with tile.TileContext(nc) as tc:
        with (
            tc.tile_pool(name="const", bufs=1) as constp,
            tc.tile_pool(name="dram", bufs=1, space="DRAM") as dram,
        ):
            if scaled:
                gs_sb = constp.tile([P, n_ht], f32)
                nc.sync.dma_start(gs_sb[:], gs_e[:].rearrange("(o p) u -> p (o u)", p=P))
                us_sb = constp.tile([P, n_ht], f32)
                nc.sync.dma_start(us_sb[:], us_e[:].rearrange("(o p) u -> p (o u)", p=P))
                n_oo = dim_shard // P
                ds_sb = constp.tile([P, n_oo], f32)
                nc.sync.dma_start(ds_sb[:], ds_e[:].rearrange("(o p) u -> p (o u)", p=P))

            hid = dram.tile([n_tc, P, n_ht, TC], bf16)
            cc_ins = [dram.tile([dim, TC], bf16, name=f"cc_in{i}")
                      for i in range(n_tc)]

            xT = xT_e[:].rearrange("(ko p) t -> p ko t", p=P)

            # ---------------- Phase A: gate/up matmuls + SwiGLU ----------------
            with (
                tc.tile_pool(name="pa", bufs=1) as pa,
                tc.tile_pool(name="psA", bufs=2, space="PSUM") as psA,
            ):
                wgu = pa.tile([P, wgu_cols], fp8, name="wgu_sb")
                q4 = wgu_cols // 4
                qeng = [nc.scalar, nc.gpsimd, nc.gpsimd, nc.scalar]
                for q in range(4):
                    qeng[q].dma_start(wgu[:, q * q4:(q + 1) * q4],
                                      wgu_e[:, q * q4:(q + 1) * q4])

                for tci in (range(n_tc) if "A" in phases else []):
                    tsl = slice(tci * TC, (tci + 1) * TC)
                    xt = pa.tile([P, n_ko, TC], bf16, tag="xt", bufs=2)
                    nc.sync.dma_start(xt[:], xT[:, :, tsl])
                    for ht in range(n_ht):
                        ps_g = psA.tile([P, TC], f32, tag="ps_g")
                        for ko in range(n_ko):
                            nc.tensor.matmul(
                                ps_g[:], wgu_sl(wgu, ht, ko, 0), xt[:, ko, :],
                                start=(ko == 0), stop=(ko == n_ko - 1),
                            )
                        ps_u = psA.tile([P, TC], f32, tag="ps_u")
                        for ko in range(n_ko):
                            nc.tensor.matmul(
                                ps_u[:], wgu_sl(wgu, ht, ko, 1), xt[:, ko, :],
                                start=(ko == 0), stop=(ko == n_ko - 1),
                            )
                        t_silu = pa.tile([P, TC], f32, tag="t_silu", bufs=2)
                        nc.scalar.activation(
                            t_silu[:], ps_g[:],
                            mybir.ActivationFunctionType.Silu,
                            scale=(gs_sb[:, ht:ht + 1] if scaled else 1.0),
                        )
                        hid_t = pa.tile([P, TC], bf16, tag="hid_t", bufs=3)
                        if scaled:
                            t_up = pa.tile([P, TC], f32, tag="t_up", bufs=2)
                            nc.vector.tensor_scalar(
                                t_up[:], ps_u[:], us_sb[:, ht:ht + 1], None,
                                mybir.AluOpType.mult,
                            )
                            nc.vector.tensor_tensor(
                                hid_t[:], t_silu[:], t_up[:],
                                mybir.AluOpType.mult,
                            )
                        else:
                            nc.vector.tensor_tensor(
                                hid_t[:], t_silu[:], ps_u[:],
                                mybir.AluOpType.mult,
                            )
                        nc.gpsimd.dma_start(hid[tci][:, ht, :], hid_t[:])

            # ---------------- Phase B: down matmul + ReduceScatter ----------------
            with (
                tc.tile_pool(name="pb", bufs=1) as pb,
                tc.tile_pool(name="psB", bufs=4, space="PSUM") as psB,
            ):
                wd = pb.tile([P, wd_cols], fp8, name="wd_sb")
                h2 = wd_cols // 2
                nc.sync.dma_start(wd[:, :h2], wd_e[:, :h2])
                nc.scalar.dma_start(wd[:, h2:], wd_e[:, h2:])

                for tci in (range(n_tc) if "B" in phases else []):
                    hb = pb.tile([P, n_ht, TC], bf16, tag="hb", bufs=2)
                    nc.sync.dma_start(hb[:], hid[tci])
                    cc_r = cc_ins[tci][:].rearrange("(g p) t -> p g t", p=P)
                    ob4 = None
                    for dt in range(n_dt):
                        ps = psB.tile([P, TC], f32, tag="ps_d")
                        for ho in range(n_ht):
                            nc.tensor.matmul(
                                ps[:], wd_sl(wd, dt, ho), hb[:, ho, :],
                                start=(ho == 0), stop=(ho == n_ht - 1),
                            )
                        if dt % 4 == 0:
                            ob4 = pb.tile([P, 4, TC], bf16, tag="ob4", bufs=2)
                        nc.scalar.copy(ob4[:, dt % 4, :], ps[:])
                        if dt % 4 == 3:
                            nc.sync.dma_start(
                                cc_r[:, dt - 3:dt + 1, :], ob4[:]
                            )
                    if with_collective:
                        nc.gpsimd.collective_compute(
                            "ReduceScatter",
                            mybir.AluOpType.add,
                            replica_groups=[list(range(NCORES))],
                            ins=[cc_ins[tci][:].opt()],
                            outs=[cc_outs[tci][:].opt()],
                        )
                    if scaled:
                        n_oo = dim_shard // P
                        rs_sb = pb.tile([P, n_oo, TC], bf16, tag="rs_sb", bufs=2)
                        nc.sync.dma_start(
                            rs_sb[:],
                            cc_outs[tci][:].rearrange("(o p) t -> p o t", p=P),
                        )
                        for oo in range(n_oo):
                            nc.vector.tensor_scalar(
                                rs_sb[:, oo, :], rs_sb[:, oo, :],
                                ds_sb[:, oo:oo + 1], None,
                                mybir.AluOpType.mult,
                            )
                        nc.scalar.dma_start(
                            out_e[tci].rearrange("(o p) t -> p o t", p=P),
                            rs_sb[:],
                        )
                    else:
                        nc.scalar.dma_start(out_e[tci], cc_outs[tci][:])

    nc.finalize()
    return nc


def _get_nc(d, t_total, h_total, dim, scaled=False, with_collective=True,
            phases="AB"):
    key = (d, t_total, h_total, dim, scaled, with_collective, phases)
    if key not in _CACHE:
        _CACHE[key] = _build(d, t_total, h_total, dim, scaled, with_collective,
                             phases)
    return _CACHE[key]


def _thresholds(*ws):
    """mean(|w|)*0.7 per matrix, computed with jnp on CPU to match the
    reference's XLA-CPU reduction rounding bit-for-bit."""
    import jax
    import jax.numpy as jnp

    cpu = jax.devices("cpu")[0]
    outs = []
    for w in ws:
        wc = jax.device_put(np.asarray(w), cpu)
        with jax.default_device(cpu):
            thr = jnp.mean(jnp.abs(wc)) * 0.7
        outs.append(np.float32(thr))
    return outs


def _ternarize_fp8(w, thr):
    wq = np.sign(w) * (np.abs(w) > thr)
    return wq.astype(np.float32).astype(FP8)


def prepare(x, gate_w, gate_scale, up_w, up_scale, down_w, down_scale):
    """Host-side prep: thresholds, ternarize->fp8, layout packing, sharding.
    Returns (nc, in_maps, (B, S, dim))."""
    x = np.asarray(x)
    gate_w = np.asarray(gate_w, dtype=np.float32)
    up_w = np.asarray(up_w, dtype=np.float32)
    down_w = np.asarray(down_w, dtype=np.float32)
    gate_scale = np.asarray(gate_scale, dtype=np.float32)
    up_scale = np.asarray(up_scale, dtype=np.float32)
    down_scale = np.asarray(down_scale, dtype=np.float32)

    B, S, d = x.shape
    t_total = B * S
    h_total = gate_w.shape[0]
    dim = down_w.shape[0]
    h_local = h_total // NCORES
    dim_shard = dim // NCORES
    n_ko = d // P
    n_ht = h_local // P
    n_dt = dim // P

    thr_g, thr_u, thr_d = _thresholds(gate_w, up_w, down_w)
    gq = _ternarize_fp8(gate_w, thr_g)   # [h_total, d] fp8
    uq = _ternarize_fp8(up_w, thr_u)
    dq = _ternarize_fp8(down_w, thr_d)   # [dim, h_total] fp8

    scaled = not (
        np.all(gate_scale == 1.0)
        and np.all(up_scale == 1.0)
        and np.all(down_scale == 1.0)
    )

    nc = _get_nc(d, t_total, h_total, dim, scaled=scaled)

    X = x.reshape(t_total, d).astype(np.float32)
    xT = np.ascontiguousarray(X.T).astype(BF16)

    in_maps = []
    for c in range(NCORES):
        hsl = slice(c * h_local, (c + 1) * h_local)
        osl = slice(c * dim_shard, (c + 1) * dim_shard)
        # wgu: [P, ((ht*n_ko + ko)*2 + m)*P + cc] = wT_m[ko*P+p, ht*P+cc]
        # build from gq/uq [h_local, d] -> [ht, cc(P), ko, p(P)]
        g4 = gq[hsl].reshape(n_ht, P, n_ko, P)
        u4 = uq[hsl].reshape(n_ht, P, n_ko, P)
        gu = np.stack([g4, u4], axis=0)            # [m, ht, cc, ko, p]
        wgu = np.ascontiguousarray(
            gu.transpose(4, 1, 3, 0, 2)            # [p, ht, ko, m, cc]
        ).reshape(P, n_ht * n_ko * 2 * P)
        # wd: [P, (dt*n_ht + ho)*P + cc] = dwT[ho*P+p, dt*P+cc]
        # from dq [dim, h_total] core cols -> [dt, cc, ho, p]
        d4 = dq[:, hsl].reshape(n_dt, P, n_ht, P)
        wd = np.ascontiguousarray(
            d4.transpose(3, 0, 2, 1)               # [p, dt, ho, cc]
        ).reshape(P, n_dt * n_ht * P)
        in_maps.append({
            "xT": xT,
            "wgu": wgu,
            "wd": wd,
            "gs": gate_scale[hsl],
            "us": up_scale[hsl],
            "ds": down_scale[osl],
        })
    return nc, in_maps, (B, S, dim)


def assemble(results, B, S, dim):
    t_total = B * S
    n_tc = t_total // TC
    dim_shard = dim // NCORES
    outT = np.empty((dim, t_total), np.float32)
    for c in range(NCORES):
        o = results[c]["out"].astype(np.float32)   # [n_tc, dim_shard, TC]
        for tci in range(n_tc):
            outT[c * dim_shard:(c + 1) * dim_shard,
                 tci * TC:(tci + 1) * TC] = o[tci]
    return np.ascontiguousarray(outT.T).reshape(B, S, dim)


def kernel(x, gate_w, gate_scale, up_w, up_scale, down_w, down_scale):
    from concourse.bass_utils import run_bass_kernel_spmd

    nc, in_maps, (B, S, dim) = prepare(
        x, gate_w, gate_scale, up_w, up_scale, down_w, down_scale
    )
    res = run_bass_kernel_spmd(nc, in_maps, list(range(NCORES)), trace=False)
    return assemble(res.results, B, S, dim)


if __name__ == "__main__":
    # small-scale structural self-test against a numpy reference
    rng = np.random.default_rng(0)
    d, t_total, h_total, dim = 512, 1024, 1024, 1024
    B, S = 2, t_total // 2
    x = rng.standard_normal((B, S, d), dtype=np.float32)
    gw = (rng.standard_normal((h_total, d), dtype=np.float32) / np.sqrt(d))
    uw = (rng.standard_normal((h_total, d), dtype=np.float32) / np.sqrt(d))
    dw = (rng.standard_normal((dim, h_total), dtype=np.float32) / np.sqrt(h_total))
    gsc = np.ones((h_total, 1), np.float32)
    usc = np.ones((h_total, 1), np.float32)
    dsc = np.ones((dim, 1), np.float32)

    def np_bitlinear(xf, w, scale):
        thr = np.abs(w).mean() * np.float32(0.7)
        wq = np.sign(w) * (np.abs(w) > thr)
        return xf @ (wq * scale).T

    Xf = x.reshape(-1, d)
    gate = np_bitlinear(Xf, gw, gsc)
    up = np_bitlinear(Xf, uw, usc)
    hidden = gate / (1 + np.exp(-gate)) * up
    exp = np_bitlinear(hidden, dw, dsc).reshape(B, S, dim)

    got = kernel(x=x, gate_w=gw, gate_scale=gsc, up_w=uw, up_scale=usc,
                 down_w=dw, down_scale=dsc)
    err = np.abs(got - exp).max() / np.abs(exp).max()
    print("rel absmax err:", err)
    print("PASS" if err < 1.2e-2 else "FAIL")


# revision 20
# speedup vs baseline: 1.0929x; 1.0929x over previous
"""BitNet MLP (ternary-quantized SwiGLU) on 8 Trainium2 NeuronCores — v2.

Strategy: tensor-parallel over hidden_dim. Weights are ternarized on the
HOST (exactly matching the reference's jnp-on-CPU threshold) and uploaded
as fp8e4m3 — exact for {-1,0,+1} — which quarters weight DRAM traffic and
removes all on-device quantization work. The tensor engine runs mixed-dtype
matmuls (fp8 stationary weights x bf16 moving activations), verified exact
and full-speed on HW.

Phase A keeps the core's entire gate+up weight set resident in SBUF
(128 KB/partition) and streams x through ONCE in 512-token chunks,
producing hidden in bf16 to DRAM (one DMA per chunk). Phase B keeps the
whole down_proj shard resident (64 KB/partition), streams hidden back, and
ReduceScatters bf16 partial sums per chunk (overlapped with the next
chunk's matmuls), writing straight to the output chunk.
"""

import sys

sys.path.insert(0, "/opt/trn_rl_repo")

import numpy as np
import ml_dtypes

BF16 = ml_dtypes.bfloat16
FP8 = ml_dtypes.float8_e4m3
NCORES = 8
P = 128
TC = 512

_CACHE = {}


def _build(d, t_total, h_total, dim, scaled=False, with_collective=True,
           phases="AB"):
    import concourse.mybir as mybir
    import concourse.tile as tile
    from concourse import bacc

    f32 = mybir.dt.float32
    bf16 = mybir.dt.bfloat16
    fp8 = mybir.dt.float8e4

    h_local = h_total // NCORES
    dim_shard = dim // NCORES
    n_ko = d // P
    n_ht = h_local // P
    n_dt = dim // P
    n_tc = t_total // TC

    assert d % P == 0 and h_local % P == 0 and dim % P == 0
    assert t_total % TC == 0 and n_dt % 4 == 0
    wgu_cols = n_ht * n_ko * 2 * P
    wd_cols = n_dt * n_ht * P
    assert wgu_cols % 4 == 0

    nc = bacc.Bacc("TRN2", target_bir_lowering=False, debug=False)

    xT_e = nc.dram_tensor("xT", [d, t_total], bf16, kind="ExternalInput")
    wgu_e = nc.dram_tensor("wgu", [P, wgu_cols], fp8, kind="ExternalInput")
    wd_e = nc.dram_tensor("wd", [P, wd_cols], fp8, kind="ExternalInput")
    gs_e = nc.dram_tensor("gs", [h_local, 1], f32, kind="ExternalInput")
    us_e = nc.dram_tensor("us", [h_local, 1], f32, kind="ExternalInput")
    ds_e = nc.dram_tensor("ds", [dim_shard, 1], f32, kind="ExternalInput")
    out_e = nc.dram_tensor("out", [n_tc, dim_shard, TC], bf16,
                           kind="ExternalOutput")

    cc_outs = [
        nc.dram_tensor(f"cc_out{i}", [dim_shard, TC], bf16, kind="Internal")
        for i in range(n_tc)
    ]

    def wgu_sl(w, ht, ko, m):
        c = ((ht * n_ko + ko) * 2 + m) * P
        return w[:, c:c + P]

    def wd_sl(w, dt, ho):
        c = (dt * n_ht + ho) * P
        return w[:, c:c + P]

    with tile.TileContext(nc) as tc:
        with (
            tc.tile_pool(name="const", bufs=1) as constp,
            tc.tile_pool(name="dram", bufs=1, space="DRAM") as dram,
        ):
            if scaled:
                gs_sb = constp.tile([P, n_ht], f32)
                nc.sync.dma_start(gs_sb[:], gs_e[:].rearrange("(o p) u -> p (o u)", p=P))
                us_sb = constp.tile([P, n_ht], f32)
                nc.sync.dma_start(us_sb[:], us_e[:].rearrange("(o p) u -> p (o u)", p=P))
                n_oo = dim_shard // P
                ds_sb = constp.tile([P, n_oo], f32)
                nc.sync.dma_start(ds_sb[:], ds_e[:].rearrange("(o p) u -> p (o u)", p=P))

            hid = dram.tile([n_tc, P, n_ht, TC], bf16)
            cc_ins = [dram.tile([dim, TC], bf16, name=f"cc_in{i}")
                      for i in range(n_tc)]

            xT = xT_e[:].rearrange("(ko p) t -> p ko t", p=P)

            # ---------------- Phase A: gate/up matmuls + SwiGLU ----------------
            with (
                tc.tile_pool(name="pa", bufs=1) as pa,
                tc.tile_pool(name="psA", bufs=2, space="PSUM") as psA,
            ):
                wgu = pa.tile([P, wgu_cols], fp8, name="wgu_sb")
                q4 = wgu_cols // 4
                qeng = [nc.scalar, nc.scalar, nc.scalar, nc.scalar]
                for q in range(4):
                    qeng[q].dma_start(wgu[:, q * q4:(q + 1) * q4],
                                      wgu_e[:, q * q4:(q + 1) * q4])

                for tci in (range(n_tc) if "A" in phases else []):
                    tsl = slice(tci * TC, (tci + 1) * TC)
                    xt = pa.tile([P, n_ko, TC], bf16, tag="xt", bufs=2)
                    nc.sync.dma_start(xt[:], xT[:, :, tsl])
                    for ht in range(n_ht):
                        ps_g = psA.tile([P, TC], f32, tag="ps_g")
                        for ko in range(n_ko):
                            nc.tensor.matmul(
                                ps_g[:], wgu_sl(wgu, ht, ko, 0), xt[:, ko, :],
                                start=(ko == 0), stop=(ko == n_ko - 1),
                            )
                        ps_u = psA.tile([P, TC], f32, tag="ps_u")
                        for ko in range(n_ko):
                            nc.tensor.matmul(
                                ps_u[:], wgu_sl(wgu, ht, ko, 1), xt[:, ko, :],
                                start=(ko == 0), stop=(ko == n_ko - 1),
                            )
                        t_silu = pa.tile([P, TC], f32, tag="t_silu", bufs=2)
                        nc.scalar.activation(
                            t_silu[:], ps_g[:],
                            mybir.ActivationFunctionType.Silu,
                            scale=(gs_sb[:, ht:ht + 1] if scaled else 1.0),
                        )
                        hid_t = pa.tile([P, TC], bf16, tag="hid_t", bufs=3)
                        if scaled:
                            t_up = pa.tile([P, TC], f32, tag="t_up", bufs=2)
                            nc.vector.tensor_scalar(
                                t_up[:], ps_u[:], us_sb[:, ht:ht + 1], None,
                                mybir.AluOpType.mult,
                            )
                            nc.vector.tensor_tensor(
                                hid_t[:], t_silu[:], t_up[:],
                                mybir.AluOpType.mult,
                            )
                        else:
                            nc.vector.tensor_tensor(
                                hid_t[:], t_silu[:], ps_u[:],
                                mybir.AluOpType.mult,
                            )
                        nc.scalar.dma_start(hid[tci][:, ht, :], hid_t[:])

            # ---------------- Phase B: down matmul + ReduceScatter ----------------
            with (
                tc.tile_pool(name="pb", bufs=1) as pb,
                tc.tile_pool(name="psB", bufs=4, space="PSUM") as psB,
            ):
                wd = pb.tile([P, wd_cols], fp8, name="wd_sb")
                h2 = wd_cols // 2
                nc.sync.dma_start(wd[:, :h2], wd_e[:, :h2])
                nc.scalar.dma_start(wd[:, h2:], wd_e[:, h2:])

                for tci in (range(n_tc) if "B" in phases else []):
                    hb = pb.tile([P, n_ht, TC], bf16, tag="hb", bufs=2)
                    nc.sync.dma_start(hb[:], hid[tci])
                    cc_r = cc_ins[tci][:].rearrange("(g p) t -> p g t", p=P)
                    ob4 = None
                    for dt in range(n_dt):
                        ps = psB.tile([P, TC], f32, tag="ps_d")
                        for ho in range(n_ht):
                            nc.tensor.matmul(
                                ps[:], wd_sl(wd, dt, ho), hb[:, ho, :],
                                start=(ho == 0), stop=(ho == n_ht - 1),
                            )
                        if dt % 4 == 0:
                            ob4 = pb.tile([P, 4, TC], bf16, tag="ob4", bufs=2)
                        nc.scalar.copy(ob4[:, dt % 4, :], ps[:])
                        if dt % 4 == 3:
                            nc.sync.dma_start(
                                cc_r[:, dt - 3:dt + 1, :], ob4[:]
                            )
                    if with_collective:
                        nc.gpsimd.collective_compute(
                            "ReduceScatter",
                            mybir.AluOpType.add,
                            replica_groups=[list(range(NCORES))],
                            ins=[cc_ins[tci][:].opt()],
                            outs=[cc_outs[tci][:].opt()],
                        )
                    if scaled:
                        n_oo = dim_shard // P
                        rs_sb = pb.tile([P, n_oo, TC], bf16, tag="rs_sb", bufs=2)
                        nc.sync.dma_start(
                            rs_sb[:],
                            cc_outs[tci][:].rearrange("(o p) t -> p o t", p=P),
                        )
                        for oo in range(n_oo):
                            nc.vector.tensor_scalar(
                                rs_sb[:, oo, :], rs_sb[:, oo, :],
                                ds_sb[:, oo:oo + 1], None,
                                mybir.AluOpType.mult,
                            )
                        nc.scalar.dma_start(
                            out_e[tci].rearrange("(o p) t -> p o t", p=P),
                            rs_sb[:],
                        )
                    else:
                        nc.scalar.dma_start(out_e[tci], cc_outs[tci][:])

    nc.finalize()
    return nc


def _get_nc(d, t_total, h_total, dim, scaled=False, with_collective=True,
            phases="AB"):
    key = (d, t_total, h_total, dim, scaled, with_collective, phases)
    if key not in _CACHE:
        _CACHE[key] = _build(d, t_total, h_total, dim, scaled, with_collective,
                             phases)
    return _CACHE[key]


def _thresholds(*ws):
    """mean(|w|)*0.7 per matrix, computed with jnp on CPU to match the
    reference's XLA-CPU reduction rounding bit-for-bit."""
    import jax
    import jax.numpy as jnp

    cpu = jax.devices("cpu")[0]
    outs = []
    for w in ws:
        wc = jax.device_put(np.asarray(w), cpu)
        with jax.default_device(cpu):
            thr = jnp.mean(jnp.abs(wc)) * 0.7
        outs.append(np.float32(thr))
    return outs


def _ternarize_fp8(w, thr):
    wq = np.sign(w) * (np.abs(w) > thr)
    return wq.astype(np.float32).astype(FP8)


def prepare(x, gate_w, gate_scale, up_w, up_scale, down_w, down_scale):
    """Host-side prep: thresholds, ternarize->fp8, layout packing, sharding.
    Returns (nc, in_maps, (B, S, dim))."""
    x = np.asarray(x)
    gate_w = np.asarray(gate_w, dtype=np.float32)
    up_w = np.asarray(up_w, dtype=np.float32)
    down_w = np.asarray(down_w, dtype=np.float32)
    gate_scale = np.asarray(gate_scale, dtype=np.float32)
    up_scale = np.asarray(up_scale, dtype=np.float32)
    down_scale = np.asarray(down_scale, dtype=np.float32)

    B, S, d = x.shape
    t_total = B * S
    h_total = gate_w.shape[0]
    dim = down_w.shape[0]
    h_local = h_total // NCORES
    dim_shard = dim // NCORES
    n_ko = d // P
    n_ht = h_local // P
    n_dt = dim // P

    thr_g, thr_u, thr_d = _thresholds(gate_w, up_w, down_w)
    gq = _ternarize_fp8(gate_w, thr_g)   # [h_total, d] fp8
    uq = _ternarize_fp8(up_w, thr_u)
    dq = _ternarize_fp8(down_w, thr_d)   # [dim, h_total] fp8

    scaled = not (
        np.all(gate_scale == 1.0)
        and np.all(up_scale == 1.0)
        and np.all(down_scale == 1.0)
    )

    nc = _get_nc(d, t_total, h_total, dim, scaled=scaled)

    X = x.reshape(t_total, d).astype(np.float32)
    xT = np.ascontiguousarray(X.T).astype(BF16)

    in_maps = []
    for c in range(NCORES):
        hsl = slice(c * h_local, (c + 1) * h_local)
        osl = slice(c * dim_shard, (c + 1) * dim_shard)
        # wgu: [P, ((ht*n_ko + ko)*2 + m)*P + cc] = wT_m[ko*P+p, ht*P+cc]
        # build from gq/uq [h_local, d] -> [ht, cc(P), ko, p(P)]
        g4 = gq[hsl].reshape(n_ht, P, n_ko, P)
        u4 = uq[hsl].reshape(n_ht, P, n_ko, P)
        gu = np.stack([g4, u4], axis=0)            # [m, ht, cc, ko, p]
        wgu = np.ascontiguousarray(
            gu.transpose(4, 1, 3, 0, 2)            # [p, ht, ko, m, cc]
        ).reshape(P, n_ht * n_ko * 2 * P)
        # wd: [P, (dt*n_ht + ho)*P + cc] = dwT[ho*P+p, dt*P+cc]
        # from dq [dim, h_total] core cols -> [dt, cc, ho, p]
        d4 = dq[:, hsl].reshape(n_dt, P, n_ht, P)
        wd = np.ascontiguousarray(
            d4.transpose(3, 0, 2, 1)               # [p, dt, ho, cc]
        ).reshape(P, n_dt * n_ht * P)
        in_maps.append({
            "xT": xT,
            "wgu": wgu,
            "wd": wd,
            "gs": gate_scale[hsl],
            "us": up_scale[hsl],
            "ds": down_scale[osl],
        })
    return nc, in_maps, (B, S, dim)


def assemble(results, B, S, dim):
    t_total = B * S
    n_tc = t_total // TC
    dim_shard = dim // NCORES
    outT = np.empty((dim, t_total), np.float32)
    for c in range(NCORES):
        o = results[c]["out"].astype(np.float32)   # [n_tc, dim_shard, TC]
        for tci in range(n_tc):
            outT[c * dim_shard:(c + 1) * dim_shard,
                 tci * TC:(tci + 1) * TC] = o[tci]
    return np.ascontiguousarray(outT.T).reshape(B, S, dim)


def kernel(x, gate_w, gate_scale, up_w, up_scale, down_w, down_scale):
    from concourse.bass_utils import run_bass_kernel_spmd

    nc, in_maps, (B, S, dim) = prepare(
        x, gate_w, gate_scale, up_w, up_scale, down_w, down_scale
    )
    res = run_bass_kernel_spmd(nc, in_maps, list(range(NCORES)), trace=False)
    return assemble(res.results, B, S, dim)


if __name__ == "__main__":
    # small-scale structural self-test against a numpy reference
    rng = np.random.default_rng(0)
    d, t_total, h_total, dim = 512, 1024, 1024, 1024
    B, S = 2, t_total // 2
    x = rng.standard_normal((B, S, d), dtype=np.float32)
    gw = (rng.standard_normal((h_total, d), dtype=np.float32) / np.sqrt(d))
    uw = (rng.standard_normal((h_total, d), dtype=np.float32) / np.sqrt(d))
    dw = (rng.standard_normal((dim, h_total), dtype=np.float32) / np.sqrt(h_total))
    gsc = np.ones((h_total, 1), np.float32)
    usc = np.ones((h_total, 1), np.float32)
    dsc = np.ones((dim, 1), np.float32)

    def np_bitlinear(xf, w, scale):
        thr = np.abs(w).mean() * np.float32(0.7)
        wq = np.sign(w) * (np.abs(w) > thr)
        return xf @ (wq * scale).T

    Xf = x.reshape(-1, d)
    gate = np_bitlinear(Xf, gw, gsc)
    up = np_bitlinear(Xf, uw, usc)
    hidden = gate / (1 + np.exp(-gate)) * up
    exp = np_bitlinear(hidden, dw, dsc).reshape(B, S, dim)

    got = kernel(x=x, gate_w=gw, gate_scale=gsc, up_w=uw, up_scale=usc,
                 down_w=dw, down_scale=dsc)
    err = np.abs(got - exp).max() / np.abs(exp).max()
    print("rel absmax err:", err)
    print("PASS" if err < 1.2e-2 else "FAIL")


# revision 27
# speedup vs baseline: 1.1258x; 1.0301x over previous
"""BitNet MLP (ternary-quantized SwiGLU) on 8 Trainium2 NeuronCores — v2.

Strategy: tensor-parallel over hidden_dim. Weights are ternarized on the
HOST (exactly matching the reference's jnp-on-CPU threshold) and uploaded
as fp8e4m3 — exact for {-1,0,+1} — which quarters weight DRAM traffic and
removes all on-device quantization work. The tensor engine runs mixed-dtype
matmuls (fp8 stationary weights x bf16 moving activations), verified exact
and full-speed on HW.

Phase A keeps the core's entire gate+up weight set resident in SBUF
(128 KB/partition) and streams x through ONCE in 512-token chunks,
producing hidden in bf16 to DRAM (one HWDGE store per 128-row tile — never
SWDGE, whose descriptor generation shares an SBUF port pair with DVE's
2-port perf mode and blocks the SwiGLU multiplies). Phase B keeps the whole
down_proj shard resident (64 KB/partition), streams hidden back, and
ReduceScatters bf16 partial sums per chunk (overlapped with the next
chunk's matmuls), writing straight to the output chunk.
"""

import sys

sys.path.insert(0, "/opt/trn_rl_repo")

import numpy as np
import ml_dtypes

BF16 = ml_dtypes.bfloat16
FP8 = ml_dtypes.float8_e4m3
NCORES = 8
P = 128
TC = 512

_CACHE = {}


def _build(d, t_total, h_total, dim, scaled=False, with_collective=True,
           phases="AB"):
    import concourse.mybir as mybir
    import concourse.tile as tile
    from concourse import bacc

    f32 = mybir.dt.float32
    bf16 = mybir.dt.bfloat16
    fp8 = mybir.dt.float8e4

    h_local = h_total // NCORES
    dim_shard = dim // NCORES
    n_ko = d // P
    n_ht = h_local // P
    n_dt = dim // P
    n_tc = t_total // TC

    assert d % P == 0 and h_local % P == 0 and dim % P == 0
    assert t_total % TC == 0 and n_dt % 4 == 0
    wgu_cols = n_ht * n_ko * 2 * P
    wd_cols = n_dt * n_ht * P
    assert wgu_cols % 4 == 0

    nc = bacc.Bacc("TRN2", target_bir_lowering=False, debug=False)

    xT_e = nc.dram_tensor("xT", [d, t_total], bf16, kind="ExternalInput")
    wgu_e = nc.dram_tensor("wgu", [P, wgu_cols], fp8, kind="ExternalInput")
    wd_e = nc.dram_tensor("wd", [P, wd_cols], fp8, kind="ExternalInput")
    gs_e = nc.dram_tensor("gs", [h_local, 1], f32, kind="ExternalInput")
    us_e = nc.dram_tensor("us", [h_local, 1], f32, kind="ExternalInput")
    ds_e = nc.dram_tensor("ds", [dim_shard, 1], f32, kind="ExternalInput")
    out_e = nc.dram_tensor("out", [n_tc, dim_shard, TC], bf16,
                           kind="ExternalOutput")

    cc_outs = [
        nc.dram_tensor(f"cc_out{i}", [dim_shard, TC], bf16, kind="Internal")
        for i in range(n_tc)
    ]

    def wgu_sl(w, ht, ko, m):
        c = ((ht * 2 + m) * n_ko + ko) * P
        return w[:, c:c + P]

    def wd_sl(w, dt, ho):
        c = (dt * n_ht + ho) * P
        return w[:, c:c + P]

    with tile.TileContext(nc) as tc:
        with (
            tc.tile_pool(name="const", bufs=1) as constp,
            tc.tile_pool(name="dram", bufs=1, space="DRAM") as dram,
        ):
            if scaled:
                gs_sb = constp.tile([P, n_ht], f32)
                nc.sync.dma_start(gs_sb[:], gs_e[:].rearrange("(o p) u -> p (o u)", p=P))
                us_sb = constp.tile([P, n_ht], f32)
                nc.sync.dma_start(us_sb[:], us_e[:].rearrange("(o p) u -> p (o u)", p=P))
                n_oo = dim_shard // P
                ds_sb = constp.tile([P, n_oo], f32)
                nc.sync.dma_start(ds_sb[:], ds_e[:].rearrange("(o p) u -> p (o u)", p=P))

            hid = dram.tile([n_tc, P, n_ht, TC], bf16)
            cc_ins = [dram.tile([dim, TC], bf16, name=f"cc_in{i}")
                      for i in range(n_tc)]

            xT = xT_e[:].rearrange("(ko p) t -> p ko t", p=P)

            # ---------------- Phase A: gate/up matmuls + SwiGLU ----------------
            with (
                tc.tile_pool(name="pa", bufs=1) as pa,
                tc.tile_pool(name="psA", bufs=2, space="PSUM") as psA,
            ):
                wgu = pa.tile([P, wgu_cols], fp8, name="wgu_sb")
                q4 = wgu_cols // 4
                qeng = [nc.scalar, nc.scalar, nc.scalar, nc.scalar]
                for q in range(4):
                    qeng[q].dma_start(wgu[:, q * q4:(q + 1) * q4],
                                      wgu_e[:, q * q4:(q + 1) * q4])

                for tci in (range(n_tc) if "A" in phases else []):
                    tsl = slice(tci * TC, (tci + 1) * TC)
                    xt = pa.tile([P, n_ko, TC], bf16, tag="xt", bufs=2)
                    nc.sync.dma_start(xt[:], xT[:, :, tsl])
                    for ht in range(n_ht):
                        ps_g = psA.tile([P, TC], f32, tag="ps_g")
                        for ko in range(n_ko):
                            nc.tensor.matmul(
                                ps_g[:], wgu_sl(wgu, ht, ko, 0), xt[:, ko, :],
                                start=(ko == 0), stop=(ko == n_ko - 1),
                            )
                        ps_u = psA.tile([P, TC], f32, tag="ps_u")
                        for ko in range(n_ko):
                            nc.tensor.matmul(
                                ps_u[:], wgu_sl(wgu, ht, ko, 1), xt[:, ko, :],
                                start=(ko == 0), stop=(ko == n_ko - 1),
                            )
                        t_silu = pa.tile([P, TC], f32, tag="t_silu", bufs=2)
                        nc.scalar.activation(
                            t_silu[:], ps_g[:],
                            mybir.ActivationFunctionType.Silu,
                            scale=(gs_sb[:, ht:ht + 1] if scaled else 1.0),
                        )
                        hid_t = pa.tile([P, TC], bf16, tag="hid_t", bufs=3)
                        if scaled:
                            t_up = pa.tile([P, TC], f32, tag="t_up", bufs=2)
                            nc.vector.tensor_scalar(
                                t_up[:], ps_u[:], us_sb[:, ht:ht + 1], None,
                                mybir.AluOpType.mult,
                            )
                            nc.vector.tensor_tensor(
                                hid_t[:], t_silu[:], t_up[:],
                                mybir.AluOpType.mult,
                            )
                        else:
                            nc.vector.tensor_tensor(
                                hid_t[:], t_silu[:], ps_u[:],
                                mybir.AluOpType.mult,
                            )
                        nc.scalar.dma_start(hid[tci][:, ht, :], hid_t[:])

            # ---------------- Phase B: down matmul + ReduceScatter ----------------
            with (
                tc.tile_pool(name="pb", bufs=1) as pb,
                tc.tile_pool(name="psB", bufs=4, space="PSUM") as psB,
            ):
                wd = pb.tile([P, wd_cols], fp8, name="wd_sb")
                h2 = wd_cols // 2
                nc.sync.dma_start(wd[:, :h2], wd_e[:, :h2])
                nc.scalar.dma_start(wd[:, h2:], wd_e[:, h2:])

                for tci in (range(n_tc) if "B" in phases else []):
                    hb = pb.tile([P, n_ht, TC], bf16, tag="hb", bufs=2)
                    nc.sync.dma_start(hb[:], hid[tci])
                    cc_r = cc_ins[tci][:].rearrange("(g p) t -> p g t", p=P)
                    ob4 = None
                    for dt in range(n_dt):
                        ps = psB.tile([P, TC], f32, tag="ps_d")
                        for ho in range(n_ht):
                            nc.tensor.matmul(
                                ps[:], wd_sl(wd, dt, ho), hb[:, ho, :],
                                start=(ho == 0), stop=(ho == n_ht - 1),
                            )
                        if dt % 4 == 0:
                            ob4 = pb.tile([P, 4, TC], bf16, tag="ob4", bufs=2)
                        nc.scalar.copy(ob4[:, dt % 4, :], ps[:])
                        if dt % 4 == 3:
                            nc.sync.dma_start(
                                cc_r[:, dt - 3:dt + 1, :], ob4[:]
                            )
                    if with_collective:
                        nc.gpsimd.collective_compute(
                            "ReduceScatter",
                            mybir.AluOpType.add,
                            replica_groups=[list(range(NCORES))],
                            ins=[cc_ins[tci][:].opt()],
                            outs=[cc_outs[tci][:].opt()],
                        )
                    if scaled:
                        n_oo = dim_shard // P
                        rs_sb = pb.tile([P, n_oo, TC], bf16, tag="rs_sb", bufs=2)
                        nc.sync.dma_start(
                            rs_sb[:],
                            cc_outs[tci][:].rearrange("(o p) t -> p o t", p=P),
                        )
                        for oo in range(n_oo):
                            nc.vector.tensor_scalar(
                                rs_sb[:, oo, :], rs_sb[:, oo, :],
                                ds_sb[:, oo:oo + 1], None,
                                mybir.AluOpType.mult,
                            )
                        nc.scalar.dma_start(
                            out_e[tci].rearrange("(o p) t -> p o t", p=P),
                            rs_sb[:],
                        )
                    else:
                        nc.scalar.dma_start(out_e[tci], cc_outs[tci][:])

    nc.finalize()
    return nc


def _get_nc(d, t_total, h_total, dim, scaled=False, with_collective=True,
            phases="AB"):
    key = (d, t_total, h_total, dim, scaled, with_collective, phases)
    if key not in _CACHE:
        _CACHE[key] = _build(d, t_total, h_total, dim, scaled, with_collective,
                             phases)
    return _CACHE[key]


def _thresholds(*ws):
    """mean(|w|)*0.7 per matrix, computed with jnp on CPU to match the
    reference's XLA-CPU reduction rounding bit-for-bit."""
    import jax
    import jax.numpy as jnp

    cpu = jax.devices("cpu")[0]
    outs = []
    for w in ws:
        wc = jax.device_put(np.asarray(w), cpu)
        with jax.default_device(cpu):
            thr = jnp.mean(jnp.abs(wc)) * 0.7
        outs.append(np.float32(thr))
    return outs


def _ternarize_fp8(w, thr):
    wq = np.sign(w) * (np.abs(w) > thr)
    return wq.astype(np.float32).astype(FP8)


def prepare(x, gate_w, gate_scale, up_w, up_scale, down_w, down_scale):
    """Host-side prep: thresholds, ternarize->fp8, layout packing, sharding.
    Returns (nc, in_maps, (B, S, dim))."""
    x = np.asarray(x)
    gate_w = np.asarray(gate_w, dtype=np.float32)
    up_w = np.asarray(up_w, dtype=np.float32)
    down_w = np.asarray(down_w, dtype=np.float32)
    gate_scale = np.asarray(gate_scale, dtype=np.float32)
    up_scale = np.asarray(up_scale, dtype=np.float32)
    down_scale = np.asarray(down_scale, dtype=np.float32)

    B, S, d = x.shape
    t_total = B * S
    h_total = gate_w.shape[0]
    dim = down_w.shape[0]
    h_local = h_total // NCORES
    dim_shard = dim // NCORES
    n_ko = d // P
    n_ht = h_local // P
    n_dt = dim // P

    thr_g, thr_u, thr_d = _thresholds(gate_w, up_w, down_w)
    gq = _ternarize_fp8(gate_w, thr_g)   # [h_total, d] fp8
    uq = _ternarize_fp8(up_w, thr_u)
    dq = _ternarize_fp8(down_w, thr_d)   # [dim, h_total] fp8

    scaled = not (
        np.all(gate_scale == 1.0)
        and np.all(up_scale == 1.0)
        and np.all(down_scale == 1.0)
    )

    nc = _get_nc(d, t_total, h_total, dim, scaled=scaled)

    X = x.reshape(t_total, d).astype(np.float32)
    xT = np.ascontiguousarray(X.T).astype(BF16)

    in_maps = []
    for c in range(NCORES):
        hsl = slice(c * h_local, (c + 1) * h_local)
        osl = slice(c * dim_shard, (c + 1) * dim_shard)
        # wgu: [P, ((ht*n_ko + ko)*2 + m)*P + cc] = wT_m[ko*P+p, ht*P+cc]
        # build from gq/uq [h_local, d] -> [ht, cc(P), ko, p(P)]
        g4 = gq[hsl].reshape(n_ht, P, n_ko, P)
        u4 = uq[hsl].reshape(n_ht, P, n_ko, P)
        gu = np.stack([g4, u4], axis=0)            # [m, ht, cc, ko, p]
        wgu = np.ascontiguousarray(
            gu.transpose(4, 1, 0, 3, 2)            # [p, ht, m, ko, cc]
        ).reshape(P, n_ht * n_ko * 2 * P)
        # wd: [P, (dt*n_ht + ho)*P + cc] = dwT[ho*P+p, dt*P+cc]
        # from dq [dim, h_total] core cols -> [dt, cc, ho, p]
        d4 = dq[:, hsl].reshape(n_dt, P, n_ht, P)
        wd = np.ascontiguousarray(
            d4.transpose(3, 0, 2, 1)               # [p, dt, ho, cc]
        ).reshape(P, n_dt * n_ht * P)
        in_maps.append({
            "xT": xT,
            "wgu": wgu,
            "wd": wd,
            "gs": gate_scale[hsl],
            "us": up_scale[hsl],
            "ds": down_scale[osl],
        })
    return nc, in_maps, (B, S, dim)


def assemble(results, B, S, dim):
    t_total = B * S
    n_tc = t_total // TC
    dim_shard = dim // NCORES
    outT = np.empty((dim, t_total), np.float32)
    for c in range(NCORES):
        o = results[c]["out"].astype(np.float32)   # [n_tc, dim_shard, TC]
        for tci in range(n_tc):
            outT[c * dim_shard:(c + 1) * dim_shard,
                 tci * TC:(tci + 1) * TC] = o[tci]
    return np.ascontiguousarray(outT.T).reshape(B, S, dim)


def kernel(x, gate_w, gate_scale, up_w, up_scale, down_w, down_scale):
    from concourse.bass_utils import run_bass_kernel_spmd

    nc, in_maps, (B, S, dim) = prepare(
        x, gate_w, gate_scale, up_w, up_scale, down_w, down_scale
    )
    res = run_bass_kernel_spmd(nc, in_maps, list(range(NCORES)), trace=False)
    return assemble(res.results, B, S, dim)


if __name__ == "__main__":
    # small-scale structural self-test against a numpy reference
    rng = np.random.default_rng(0)
    d, t_total, h_total, dim = 512, 1024, 1024, 1024
    B, S = 2, t_total // 2
    x = rng.standard_normal((B, S, d), dtype=np.float32)
    gw = (rng.standard_normal((h_total, d), dtype=np.float32) / np.sqrt(d))
    uw = (rng.standard_normal((h_total, d), dtype=np.float32) / np.sqrt(d))
    dw = (rng.standard_normal((dim, h_total), dtype=np.float32) / np.sqrt(h_total))
    gsc = np.ones((h_total, 1), np.float32)
    usc = np.ones((h_total, 1), np.float32)
    dsc = np.ones((dim, 1), np.float32)

    def np_bitlinear(xf, w, scale):
        thr = np.abs(w).mean() * np.float32(0.7)
        wq = np.sign(w) * (np.abs(w) > thr)
        return xf @ (wq * scale).T

    Xf = x.reshape(-1, d)
    gate = np_bitlinear(Xf, gw, gsc)
    up = np_bitlinear(Xf, uw, usc)
    hidden = gate / (1 + np.exp(-gate)) * up
    exp = np_bitlinear(hidden, dw, dsc).reshape(B, S, dim)

    got = kernel(x=x, gate_w=gw, gate_scale=gsc, up_w=uw, up_scale=usc,
                 down_w=dw, down_scale=dsc)
    err = np.abs(got - exp).max() / np.abs(exp).max()
    print("rel absmax err:", err)
    print("PASS" if err < 1.2e-2 else "FAIL")
